# revision 11
# baseline (speedup 1.0000x reference)
"""TRN2 Bass kernel for nn_Attention_23493471109551.

Full attention layer: QKV projections + interleaved RoPE + causal softmax
attention + output projection, for B=4, S=2048, D=1024, H=16, Dh=64, fp32.

Sharding: 8 cores = 4 batches x 2 head-groups (8 heads each).  Each core
computes its batch/head-group's attention and a partial output projection
(W_o row-block); host sums the two partials per batch.

All matmuls run in fp32r (fp32 rounded to 11-bit mantissa, 1 cyc/row on the
PE at N>=512 vs 4 for fp32).  Inputs are pre-rounded on host; on-device
intermediates are rounded by the producing engine writing float32r tiles.

Layout strategy (per core):
  qpT/kpT: [dh-on-partitions, S]  (heads stacked 2-per-128-partitions)
  scores computed TRANSPOSED [sk, sq] so probs feed the PV matmul directly
  (no per-tile transposes); softmax denominator comes free as a ones-column
  appended to V (fused into the PV matmul, normalized once per [65,512]
  output block); causal mask is an additive -1e30 on the score PSUM applied
  only to the diagonal 128x128 blocks, with score/exp/PV column ranges
  trimmed to the causal triangle; RoPE pair-mixing uses a DVE stream-shuffle.

Measured (8 cores, NTFF profile): ~800-880 us per core, rel err ~1.8e-4.
"""
import math
import numpy as np

import concourse.bass as bass
import concourse.tile as tile
import concourse.mybir as mybir
from concourse import bacc, bass_utils

# problem constants
B, S, D = 4, 2048, 1024
H, Dh = 16, 64
EQ, EV = 2048, 1024          # q/k and v input feature dims
F = 512                      # features per core (8 heads x 64)
P = 128
N_CORES = 8
SCALE = 1.0 / math.sqrt(D)   # 1/32
ROPE_BASE = 10000.0
NEG = -1.0e30
SWAP_MASK = [i ^ 1 for i in range(32)]

F32 = mybir.dt.float32
F32R = mybir.dt.float32r

# test hooks (harness ignores these)
KERNEL_TRACE = False
LAST_RESULT = None

_nc_cache = None


def _round_fp32r(x: np.ndarray) -> np.ndarray:
    """Round fp32 array to the fp32r grid (11-bit mantissa, RNE)."""
    u = np.ascontiguousarray(x, dtype=np.float32).view(np.uint32)
    low = u & np.uint32(0xFFF)
    hi = u >> np.uint32(12)
    round_up = (low > np.uint32(0x800)) | (
        (low == np.uint32(0x800)) & ((hi & np.uint32(1)) == 1)
    )
    hi = hi + round_up.astype(np.uint32)
    return (hi << np.uint32(12)).view(np.float32)


def _build_nc():
    nc = bacc.Bacc("TRN2", target_bir_lowering=False, debug=False)
    qT = nc.dram_tensor("qT", [EQ, S], F32R, kind="ExternalInput").ap()
    kT = nc.dram_tensor("kT", [EQ, S], F32R, kind="ExternalInput").ap()
    vT = nc.dram_tensor("vT", [EV, S], F32R, kind="ExternalInput").ap()
    wqT = nc.dram_tensor("wqT", [EQ, F], F32R, kind="ExternalInput").ap()
    wkT = nc.dram_tensor("wkT", [EQ, F], F32R, kind="ExternalInput").ap()
    wvT = nc.dram_tensor("wvT", [EV, F], F32R, kind="ExternalInput").ap()
    woT = nc.dram_tensor("woT", [F, D], F32R, kind="ExternalInput").ap()
    cosf = nc.dram_tensor("cosf", [P, S], F32, kind="ExternalInput").ap()
    sinf = nc.dram_tensor("sinf", [P, S], F32, kind="ExternalInput").ap()
    maskA = nc.dram_tensor("maskA", [P, P], F32, kind="ExternalInput").ap()
    out = nc.dram_tensor("out", [S, D], F32, kind="ExternalOutput").ap()

    EXP = mybir.ActivationFunctionType.Exp

    with tile.TileContext(nc) as tc:
        with (
            tc.tile_pool(name="consts", bufs=1) as consts,
            tc.tile_pool(name="persist", bufs=1) as persist,
            tc.tile_pool(name="qt", bufs=3) as qt_pool,
            tc.tile_pool(name="wsmall", bufs=4) as w_pool,
            tc.tile_pool(name="rope", bufs=2) as rope_pool,
            tc.tile_pool(name="expp", bufs=3) as exp_pool,
            tc.tile_pool(name="norm", bufs=2) as norm_pool,
            tc.tile_pool(name="outsb", bufs=3) as out_pool,
            tc.tile_pool(name="attnc", bufs=2) as attnc_pool,
            tc.tile_pool(name="ps1", bufs=4, space="PSUM") as ps1,
            tc.tile_pool(name="ps2", bufs=2, space="PSUM") as ps2,
        ):
            # ---- persistent activations
            qpT = persist.tile([P, 4, S], F32R, tag="qpT")
            kpT = persist.tile([P, 4, S], F32R, tag="kpT")
            vpa = persist.tile([P, 16, 8, 65], F32R, tag="vpa")

            nc.vector.memset(vpa[:].bitcast(F32), 1.0)  # ones col; 0:64 overwritten

            # ---- constants (emitted after the first proj tiles get queue priority)
            cos_t = consts.tile([P, S], F32, tag="cos")
            sin_t = consts.tile([P, S], F32, tag="sin")
            mask_t = consts.tile([P, P], F32, tag="mask")
            wo_t = consts.tile([P, 4, D], F32R, tag="woT")

            # ---- q/k projections + rope (qpT[f, s] = sum_e WT[e,f] * xT[e,s])
            _const_dmas_emitted = False
            for src, wsrc, dstT in ((qT, wqT, qpT), (kT, wkT, kpT)):
                for sc in range(4):
                    ssl = slice(sc * 512, (sc + 1) * 512)
                    psums = [ps1.tile([P, 512], F32, tag="b1", name=f"psq{i}") for i in range(4)]
                    for e in range(16):
                        at = qt_pool.tile([P, 512], F32R, tag="qt")
                        nc.sync.dma_start(at[:], src[e * P:(e + 1) * P, ssl])
                        wt = w_pool.tile([P, F], F32R, tag="w")
                        nc.sync.dma_start(wt[:], wsrc[e * P:(e + 1) * P, :])
                        for ci in range(4):
                            nc.tensor.matmul(psums[ci][:], wt[:, ci * P:(ci + 1) * P],
                                             at[:], start=(e == 0), stop=(e == 15))
                    if not _const_dmas_emitted:
                        _const_dmas_emitted = True
                        nc.sync.dma_start(cos_t[:], cosf)
                        nc.sync.dma_start(sin_t[:], sinf)
                        nc.sync.dma_start(mask_t[:], maskA)
                        for _ci in range(4):
                            nc.sync.dma_start(wo_t[:, _ci, :],
                                              woT[_ci * P:(_ci + 1) * P, :])
                    # rope: out = x*cos + pairswap(x*sin')
                    for ci in range(4):
                        a_t = rope_pool.tile([P, 512], F32, tag="ropeA")
                        nc.vector.tensor_mul(a_t[:], psums[ci][:], cos_t[:, ssl])
                        c_t = rope_pool.tile([P, 512], F32, tag="ropeC")
                        nc.vector.tensor_mul(c_t[:], psums[ci][:], sin_t[:, ssl])
                        c2_t = rope_pool.tile([P, 512], F32, tag="ropeC")
                        nc.vector.stream_shuffle(c2_t[:], c_t[:], SWAP_MASK)
                        nc.vector.tensor_add(dstT[:, ci, ssl], a_t[:], c2_t[:])

            # ---- v projection (vp[s, f] = sum_e vT[e,s] * wvT[e,f])
            for stq in range(4):
                psv = [ps1.tile([P, 512], F32, tag="b1", name=f"psv{i}") for i in range(4)]
                for e in range(8):
                    wvt = w_pool.tile([P, F], F32R, tag="w")
                    nc.sync.dma_start(wvt[:], wvT[e * P:(e + 1) * P, :])
                    vt = w_pool.tile([P, F], F32R, tag="w", name="vtrow")
                    nc.sync.dma_start(
                        vt[:], vT[e * P:(e + 1) * P, stq * 512:(stq + 1) * 512])
                    for j in range(4):
                        nc.tensor.matmul(psv[j][:], vt[:, j * P:(j + 1) * P], wvt[:],
                                         start=(e == 0), stop=(e == 7))
                for j in range(4):
                    st = stq * 4 + j
                    nc.scalar.copy(vpa[:, st, :, 0:64],
                                   psv[j][:].rearrange("p (h d) -> p h d", h=8))

            # ---- attention (scoresT layout) + output projection, per s-chunk
            for c in range(4):
                ssl = slice(c * 512, (c + 1) * 512)
                nt = 4 * (c + 1)
                attn_c = attnc_pool.tile([P, 4, 512], F32R, tag="attn_c")
                for hp in range(4):
                    po_a = ps1.tile([P, 512], F32, tag="b1")
                    po_b = ps1.tile([P, 512], F32, tag="b1")
                    for t in range(nt):
                        tsl = slice(t * P, (t + 1) * P)
                        rr = 128 * (t - 4 * c) if t >= 4 * c else 0
                        qsl = slice(c * 512 + rr, (c + 1) * 512)
                        ps_s = ps2.tile([P, 1024], F32, tag="b2")
                        nc.tensor.matmul(ps_s[:, rr:512], kpT[0:64, hp, tsl],
                                         qpT[0:64, hp, qsl], start=True, stop=True)
                        nc.tensor.matmul(ps_s[:, 512 + rr:1024], kpT[64:128, hp, tsl],
                                         qpT[64:128, hp, qsl], start=True, stop=True)
                        sv = ps_s[:].rearrange("p (two n) -> p two n", two=2)
                        if t >= 4 * c:
                            nc.vector.tensor_add(
                                sv[:, :, rr:rr + 128], sv[:, :, rr:rr + 128],
                                mask_t[:, None, :].to_broadcast((P, 2, P)))
                        et = exp_pool.tile([P, 2, 512], F32R, tag="exp")
                        nc.scalar.activation(et[:, :, rr:512], sv[:, :, rr:512],
                                             EXP, scale=SCALE)
                        nc.tensor.matmul(po_a[0:65, rr:512], vpa[:, t, 2 * hp, :],
                                         et[:, 0, rr:512], start=(t == 0), stop=(t == nt - 1))
                        nc.tensor.matmul(po_b[0:65, rr:512], vpa[:, t, 2 * hp + 1, :],
                                         et[:, 1, rr:512], start=(t == 0), stop=(t == nt - 1))
                    # normalize by the ones-column sum (row 64)
                    for half, po in ((0, po_a), (1, po_b)):
                        posb = norm_pool.tile([65, 512], F32, tag="posb")
                        nc.scalar.copy(posb[:], po[0:65, :])
                        rc = norm_pool.tile([1, 512], F32, tag="recip")
                        nc.vector.reciprocal(rc[:], posb[64:65, :])
                        bc = norm_pool.tile([64, 512], F32, tag="bcast")
                        nc.gpsimd.partition_broadcast(bc[:], rc[:])
                        nc.vector.tensor_mul(
                            attn_c[64 * half:64 * (half + 1), hp, :],
                            posb[0:64, :], bc[:])
                # W_o for this chunk's 4 s-tiles
                for j in range(4):
                    pw = [ps1.tile([P, 512], F32, tag="b1", name=f"pw{i}") for i in range(2)]
                    for ci in range(4):
                        for oc in range(2):
                            nc.tensor.matmul(pw[oc][:], attn_c[:, ci, j * P:(j + 1) * P],
                                             wo_t[:, ci, oc * 512:(oc + 1) * 512],
                                             start=(ci == 0), stop=(ci == 3))
                    row = (4 * c + j) * P
                    for oc in range(2):
                        ot = out_pool.tile([P, 512], F32, tag="osb")
                        nc.vector.tensor_copy(ot[:], pw[oc][:])
                        nc.sync.dma_start(out[row:row + P, oc * 512:(oc + 1) * 512],
                                          ot[:])
    nc.compile()
    return nc


def _tables():
    inv = (1.0 / (ROPE_BASE ** (np.arange(0, Dh, 2, dtype=np.float32) / Dh))
           ).astype(np.float32)                      # [32]
    pos = np.arange(S, dtype=np.float32)
    ang = pos[:, None] * inv[None, :]                # [S, 32]
    cos = np.cos(ang).astype(np.float32)
    sin = np.sin(ang).astype(np.float32)
    d = np.arange(P) % Dh
    i = d // 2
    cosf = np.ascontiguousarray(cos[:, i].T)         # [128, S]
    sgn = np.where(d % 2 == 0, 1.0, -1.0).astype(np.float32)
    sinf = np.ascontiguousarray(sin[:, i].T * sgn[:, None]).astype(np.float32)

    p = np.arange(P)
    j = np.arange(P)
    maskA = np.where(p[:, None] <= j[None, :], 0.0, NEG).astype(np.float32)
    return cosf, sinf, maskA


def kernel(q, k, v, W_q, W_k, W_v, W_o):
    global _nc_cache, LAST_RESULT
    if _nc_cache is None:
        _nc_cache = _build_nc()
    nc = _nc_cache

    cosf, sinf, maskA = _tables()
    q = np.asarray(q, dtype=np.float32)
    k = np.asarray(k, dtype=np.float32)
    v = np.asarray(v, dtype=np.float32)
    W_q = np.asarray(W_q, dtype=np.float32)
    W_k = np.asarray(W_k, dtype=np.float32)
    W_v = np.asarray(W_v, dtype=np.float32)
    W_o = np.asarray(W_o, dtype=np.float32)

    in_maps = []
    for b in range(B):
        qTb = _round_fp32r(q[b].T)
        kTb = _round_fp32r(k[b].T)
        vTb = _round_fp32r(v[b].T)
        for g in range(2):
            fs = slice(g * F, (g + 1) * F)
            in_maps.append({
                "qT": qTb, "kT": kTb, "vT": vTb,
                "wqT": _round_fp32r(W_q[fs, :].T),
                "wkT": _round_fp32r(W_k[fs, :].T),
                "wvT": _round_fp32r(W_v[fs, :].T),
                "woT": _round_fp32r(W_o[:, fs].T),
                "cosf": cosf, "sinf": sinf, "maskA": maskA,
            })

    res = bass_utils.run_bass_kernel_spmd(
        nc, in_maps, core_ids=list(range(N_CORES)), trace=KERNEL_TRACE)
    LAST_RESULT = res

    final = np.empty((B, S, D), dtype=np.float32)
    for b in range(B):
        final[b] = res.results[2 * b]["out"] + res.results[2 * b + 1]["out"]
    return final


# revision 12
# speedup vs baseline: 1.1810x; 1.1810x over previous
"""TRN2 Bass kernel for nn_Attention_23493471109551.

Full attention layer: QKV projections + interleaved RoPE + causal softmax
attention + output projection, for B=4, S=2048, D=1024, H=16, Dh=64, fp32.

Sharding: 8 cores = 4 batches x 2 head-groups (8 heads each).  Each core
computes its batch/head-group's attention and a partial output projection
(W_o row-block); host sums the two partials per batch.

All matmuls run in fp32r (fp32 rounded to 11-bit mantissa, 1 cyc/row on the
PE at N>=512 vs 4 for fp32).  Inputs are pre-rounded on host; on-device
intermediates are rounded by the producing engine writing float32r tiles.

Layout strategy (per core):
  qpT/kpT: [dh-on-partitions, S]  (heads stacked 2-per-128-partitions)
  scores computed TRANSPOSED [sk, sq] so probs feed the PV matmul directly
  (no per-tile transposes); softmax denominator comes free as a ones-column
  appended to V (fused into the PV matmul, normalized once per [65,512]
  output block); causal mask is an additive -1e30 on the score PSUM applied
  only to the diagonal 128x128 blocks, with score/exp/PV column ranges
  trimmed to the causal triangle; RoPE pair-mixing uses a DVE stream-shuffle.

Measured (8 cores, NTFF profile): ~800-880 us per core, rel err ~1.8e-4.
"""
import math
import numpy as np

import concourse.bass as bass
import concourse.tile as tile
import concourse.mybir as mybir
from concourse import bacc, bass_utils

# problem constants
B, S, D = 4, 2048, 1024
H, Dh = 16, 64
EQ, EV = 2048, 1024          # q/k and v input feature dims
F = 512                      # features per core (8 heads x 64)
P = 128
N_CORES = 8
SCALE = 1.0 / math.sqrt(D)   # 1/32
ROPE_BASE = 10000.0
NEG = -1.0e30
SWAP_MASK = [i ^ 1 for i in range(32)]

F32 = mybir.dt.float32
F32R = mybir.dt.float32r

# test hooks (harness ignores these)
KERNEL_TRACE = False
LAST_RESULT = None

_nc_cache = None


def _round_fp32r(x: np.ndarray) -> np.ndarray:
    """Round fp32 array to the fp32r grid (11-bit mantissa, RNE)."""
    u = np.ascontiguousarray(x, dtype=np.float32).view(np.uint32)
    low = u & np.uint32(0xFFF)
    hi = u >> np.uint32(12)
    round_up = (low > np.uint32(0x800)) | (
        (low == np.uint32(0x800)) & ((hi & np.uint32(1)) == 1)
    )
    hi = hi + round_up.astype(np.uint32)
    return (hi << np.uint32(12)).view(np.float32)


def _build_nc():
    nc = bacc.Bacc("TRN2", target_bir_lowering=False, debug=False)
    qT = nc.dram_tensor("qT", [EQ, S], F32R, kind="ExternalInput").ap()
    kT = nc.dram_tensor("kT", [EQ, S], F32R, kind="ExternalInput").ap()
    vT = nc.dram_tensor("vT", [EV, S], F32R, kind="ExternalInput").ap()
    wqT = nc.dram_tensor("wqT", [EQ, F], F32R, kind="ExternalInput").ap()
    wkT = nc.dram_tensor("wkT", [EQ, F], F32R, kind="ExternalInput").ap()
    wvT = nc.dram_tensor("wvT", [EV, F], F32R, kind="ExternalInput").ap()
    woT = nc.dram_tensor("woT", [F, D], F32R, kind="ExternalInput").ap()
    cosf = nc.dram_tensor("cosf", [P, S], F32, kind="ExternalInput").ap()
    sinf = nc.dram_tensor("sinf", [P, S], F32, kind="ExternalInput").ap()
    maskA = nc.dram_tensor("maskA", [P, P], F32, kind="ExternalInput").ap()
    out = nc.dram_tensor("out", [S, D], F32, kind="ExternalOutput").ap()

    EXP = mybir.ActivationFunctionType.Exp

    with tile.TileContext(nc) as tc:
        with (
            tc.tile_pool(name="consts", bufs=1) as consts,
            tc.tile_pool(name="persist", bufs=1) as persist,
            tc.tile_pool(name="qt", bufs=5) as qt_pool,
            tc.tile_pool(name="wsmall", bufs=6) as w_pool,
            tc.tile_pool(name="rope", bufs=2) as rope_pool,
            tc.tile_pool(name="expp", bufs=3) as exp_pool,
            tc.tile_pool(name="norm", bufs=2) as norm_pool,
            tc.tile_pool(name="outsb", bufs=3) as out_pool,
            tc.tile_pool(name="attnc", bufs=2) as attnc_pool,
            tc.tile_pool(name="ps1", bufs=4, space="PSUM") as ps1,
            tc.tile_pool(name="ps2", bufs=2, space="PSUM") as ps2,
        ):
            # ---- persistent activations
            qpT = persist.tile([P, 4, S], F32R, tag="qpT")
            kpT = persist.tile([P, 4, S], F32R, tag="kpT")
            vpa = persist.tile([P, 16, 8, 65], F32R, tag="vpa")

            nc.vector.memset(vpa[:].bitcast(F32), 1.0)  # ones col; 0:64 overwritten

            # ---- constants (emitted after the first proj tiles get queue priority)
            cos_t = consts.tile([P, S], F32, tag="cos")
            sin_t = consts.tile([P, S], F32, tag="sin")
            mask_t = consts.tile([P, P], F32, tag="mask")
            wo_t = consts.tile([P, 4, D], F32R, tag="woT")

            # ---- q/k projections + rope (qpT[f, s] = sum_e WT[e,f] * xT[e,s])
            _const_dmas_emitted = False
            for src, wsrc, dstT in ((qT, wqT, qpT), (kT, wkT, kpT)):
                for sc in range(4):
                    ssl = slice(sc * 512, (sc + 1) * 512)
                    psums = [None] * 4
                    for e in range(16):
                        at = qt_pool.tile([P, 512], F32R, tag="qt")
                        nc.sync.dma_start(at[:], src[e * P:(e + 1) * P, ssl])
                        wt = w_pool.tile([P, F], F32R, tag="w")
                        nc.sync.dma_start(wt[:], wsrc[e * P:(e + 1) * P, :])
                        for ci in range(4):
                            if psums[ci] is None:
                                psums[ci] = ps1.tile([P, 512], F32, tag="b1",
                                                     name=f"psq{ci}")
                            nc.tensor.matmul(psums[ci][:], wt[:, ci * P:(ci + 1) * P],
                                             at[:], start=(e == 0), stop=(e == 15))
                    if not _const_dmas_emitted:
                        _const_dmas_emitted = True
                        nc.sync.dma_start(cos_t[:], cosf)
                        nc.sync.dma_start(sin_t[:], sinf)
                        nc.sync.dma_start(mask_t[:], maskA)
                        for _ci in range(4):
                            nc.sync.dma_start(wo_t[:, _ci, :],
                                              woT[_ci * P:(_ci + 1) * P, :])
                    # rope: out = x*cos + pairswap(x*sin')
                    for ci in range(4):
                        a_t = rope_pool.tile([P, 512], F32, tag="ropeA")
                        nc.vector.tensor_mul(a_t[:], psums[ci][:], cos_t[:, ssl])
                        c_t = rope_pool.tile([P, 512], F32, tag="ropeC")
                        nc.vector.tensor_mul(c_t[:], psums[ci][:], sin_t[:, ssl])
                        c2_t = rope_pool.tile([P, 512], F32, tag="ropeC")
                        nc.vector.stream_shuffle(c2_t[:], c_t[:], SWAP_MASK)
                        nc.vector.tensor_add(dstT[:, ci, ssl], a_t[:], c2_t[:])

            # ---- v projection (vp[s, f] = sum_e vT[e,s] * wvT[e,f])
            for stq in range(4):
                psv = [ps1.tile([P, 512], F32, tag="b1", name=f"psv{i}") for i in range(4)]
                for e in range(8):
                    wvt = w_pool.tile([P, F], F32R, tag="w")
                    nc.sync.dma_start(wvt[:], wvT[e * P:(e + 1) * P, :])
                    vt = w_pool.tile([P, F], F32R, tag="w", name="vtrow")
                    nc.sync.dma_start(
                        vt[:], vT[e * P:(e + 1) * P, stq * 512:(stq + 1) * 512])
                    for j in range(4):
                        nc.tensor.matmul(psv[j][:], vt[:, j * P:(j + 1) * P], wvt[:],
                                         start=(e == 0), stop=(e == 7))
                for j in range(4):
                    st = stq * 4 + j
                    nc.scalar.copy(vpa[:, st, :, 0:64],
                                   psv[j][:].rearrange("p (h d) -> p h d", h=8))

            # ---- attention (scoresT layout) + output projection, per s-chunk
            for c in range(4):
                ssl = slice(c * 512, (c + 1) * 512)
                nt = 4 * (c + 1)
                attn_c = attnc_pool.tile([P, 4, 512], F32R, tag="attn_c")
                for hp in range(4):
                    po_a = ps1.tile([P, 512], F32, tag="b1")
                    po_b = ps1.tile([P, 512], F32, tag="b1")
                    for t in range(nt):
                        tsl = slice(t * P, (t + 1) * P)
                        rr = 128 * (t - 4 * c) if t >= 4 * c else 0
                        qsl = slice(c * 512 + rr, (c + 1) * 512)
                        ps_s = ps2.tile([P, 1024], F32, tag="b2")
                        nc.tensor.matmul(ps_s[:, rr:512], kpT[0:64, hp, tsl],
                                         qpT[0:64, hp, qsl], start=True, stop=True)
                        nc.tensor.matmul(ps_s[:, 512 + rr:1024], kpT[64:128, hp, tsl],
                                         qpT[64:128, hp, qsl], start=True, stop=True)
                        sv = ps_s[:].rearrange("p (two n) -> p two n", two=2)
                        if t >= 4 * c:
                            nc.vector.tensor_add(
                                sv[:, :, rr:rr + 128], sv[:, :, rr:rr + 128],
                                mask_t[:, None, :].to_broadcast((P, 2, P)))
                        et = exp_pool.tile([P, 2, 512], F32R, tag="exp")
                        nc.scalar.activation(et[:, :, rr:512], sv[:, :, rr:512],
                                             EXP, scale=SCALE)
                        nc.tensor.matmul(po_a[0:65, rr:512], vpa[:, t, 2 * hp, :],
                                         et[:, 0, rr:512], start=(t == 0), stop=(t == nt - 1))
                        nc.tensor.matmul(po_b[0:65, rr:512], vpa[:, t, 2 * hp + 1, :],
                                         et[:, 1, rr:512], start=(t == 0), stop=(t == nt - 1))
                    # normalize by the ones-column sum (row 64)
                    for half, po in ((0, po_a), (1, po_b)):
                        posb = norm_pool.tile([65, 512], F32, tag="posb")
                        nc.scalar.copy(posb[:], po[0:65, :])
                        rc = norm_pool.tile([1, 512], F32, tag="recip")
                        nc.vector.reciprocal(rc[:], posb[64:65, :])
                        bc = norm_pool.tile([64, 512], F32, tag="bcast")
                        nc.gpsimd.partition_broadcast(bc[:], rc[:])
                        nc.vector.tensor_mul(
                            attn_c[64 * half:64 * (half + 1), hp, :],
                            posb[0:64, :], bc[:])
                # W_o for this chunk's 4 s-tiles
                for j in range(4):
                    pw = [ps1.tile([P, 512], F32, tag="b1", name=f"pw{i}") for i in range(2)]
                    for ci in range(4):
                        for oc in range(2):
                            nc.tensor.matmul(pw[oc][:], attn_c[:, ci, j * P:(j + 1) * P],
                                             wo_t[:, ci, oc * 512:(oc + 1) * 512],
                                             start=(ci == 0), stop=(ci == 3))
                    row = (4 * c + j) * P
                    for oc in range(2):
                        ot = out_pool.tile([P, 512], F32, tag="osb")
                        nc.vector.tensor_copy(ot[:], pw[oc][:])
                        nc.sync.dma_start(out[row:row + P, oc * 512:(oc + 1) * 512],
                                          ot[:])
    nc.compile()
    return nc


def _tables():
    inv = (1.0 / (ROPE_BASE ** (np.arange(0, Dh, 2, dtype=np.float32) / Dh))
           ).astype(np.float32)                      # [32]
    pos = np.arange(S, dtype=np.float32)
    ang = pos[:, None] * inv[None, :]                # [S, 32]
    cos = np.cos(ang).astype(np.float32)
    sin = np.sin(ang).astype(np.float32)
    d = np.arange(P) % Dh
    i = d // 2
    cosf = np.ascontiguousarray(cos[:, i].T)         # [128, S]
    sgn = np.where(d % 2 == 0, 1.0, -1.0).astype(np.float32)
    sinf = np.ascontiguousarray(sin[:, i].T * sgn[:, None]).astype(np.float32)

    p = np.arange(P)
    j = np.arange(P)
    maskA = np.where(p[:, None] <= j[None, :], 0.0, NEG).astype(np.float32)
    return cosf, sinf, maskA


def kernel(q, k, v, W_q, W_k, W_v, W_o):
    global _nc_cache, LAST_RESULT
    if _nc_cache is None:
        _nc_cache = _build_nc()
    nc = _nc_cache

    cosf, sinf, maskA = _tables()
    q = np.asarray(q, dtype=np.float32)
    k = np.asarray(k, dtype=np.float32)
    v = np.asarray(v, dtype=np.float32)
    W_q = np.asarray(W_q, dtype=np.float32)
    W_k = np.asarray(W_k, dtype=np.float32)
    W_v = np.asarray(W_v, dtype=np.float32)
    W_o = np.asarray(W_o, dtype=np.float32)

    in_maps = []
    for b in range(B):
        qTb = _round_fp32r(q[b].T)
        kTb = _round_fp32r(k[b].T)
        vTb = _round_fp32r(v[b].T)
        for g in range(2):
            fs = slice(g * F, (g + 1) * F)
            in_maps.append({
                "qT": qTb, "kT": kTb, "vT": vTb,
                "wqT": _round_fp32r(W_q[fs, :].T),
                "wkT": _round_fp32r(W_k[fs, :].T),
                "wvT": _round_fp32r(W_v[fs, :].T),
                "woT": _round_fp32r(W_o[:, fs].T),
                "cosf": cosf, "sinf": sinf, "maskA": maskA,
            })

    res = bass_utils.run_bass_kernel_spmd(
        nc, in_maps, core_ids=list(range(N_CORES)), trace=KERNEL_TRACE)
    LAST_RESULT = res

    final = np.empty((B, S, D), dtype=np.float32)
    for b in range(B):
        final[b] = res.results[2 * b]["out"] + res.results[2 * b + 1]["out"]
    return final
